# revision 39
# baseline (speedup 1.0000x reference)
"""DiscreteContinuousConv2d (sparse gnn-style conv) Trainium2 kernel.

Math: y[b,o,n] = bias[o] + sum_e psi[e] * sum_c W[o,c,k_e] * xq[in_e, b, c]
      (edges e with out_e == n), xq[i,b,c] = x[b,c,i] * qw[i].

Strategy (8 NeuronCores, output sharded — no collectives):
  - Each core owns 2048 output points = 16 blocks of 128.
  - Host sorts edges by (core, block, k); pads each (block, k) group to a
    multiple of 128 ("tiles"), identical tile counts across cores (SPMD).
  - Per block on device:
      * dma_gather: fetch the (b,c) row of xq (256 bf16 = 512 B) for every
        edge of the block; row lands on the edge's partition.
      * scatter-add as one-hot matmul: z_k^T[(b-half,c), n] += G_half.T @ S
        where S[e, n] = psi_e * one_hot(out_loc_e). Accumulates in PSUM.
      * W contraction: y^T_half += BW_k.T @ z_k^T with BW_k the
        block-diagonal (over the 2 b's of a half) W_k^T. Accumulates in PSUM.
  - Host reassembles y from the per-core (block, p, n) outputs.
"""

import numpy as np
import ml_dtypes

import bass_rust
import concourse.bass as bass
from concourse import mybir, library_config
from concourse.bass_utils import run_bass_kernel_spmd
from concourse.library_overlay import lower_extended_insts
from concourse.tile import TileContext

B, CIN, COUT, K = 4, 64, 64, 9
N_IN = N_OUT = 16384
NCORES = 8
PPC = N_OUT // NCORES          # output points per core (2048)
NBLK = PPC // 128              # blocks per core (16)
ROW = B * CIN                  # gathered row width (256)

def _prepare(x, psi_idx, psi_vals, quadrature_weights, weight):
    """Host-side sharding/sorting. Returns per-core input maps + structure."""
    bf16 = ml_dtypes.bfloat16

    xq = (x * quadrature_weights[None, None, :]).transpose(2, 0, 1)  # (n, b, c)
    XQ = np.ascontiguousarray(xq.reshape(N_IN, ROW)).astype(bf16)

    k_idx = psi_idx[0].astype(np.int64)
    out_idx = psi_idx[1].astype(np.int64)
    in_idx = psi_idx[2].astype(np.int64)

    core = out_idx // PPC
    blk = (out_idx % PPC) // 128
    loc = out_idx % 128
    gid = (core * NBLK + blk) * K + k_idx          # group id, (core, blk, k)

    order = np.argsort(gid, kind="stable")
    gid_s = gid[order]
    in_s = in_idx[order]
    loc_s = loc[order]
    psi_s = psi_vals[order]

    counts = np.bincount(gid_s, minlength=NCORES * NBLK * K).reshape(NCORES, NBLK, K)
    # tiles per (blk, k): shared across cores -> max
    T_bk = -(-counts.max(axis=0) // 128)           # (NBLK, K) ceil
    T_blk = T_bk.sum(axis=1)                       # (NBLK,)
    SLOTS_BLK = T_blk * 128
    blk_base = np.concatenate([[0], np.cumsum(SLOTS_BLK)])  # slot offset per blk
    SLOTS = int(blk_base[-1])                      # total slots per core

    # slot offset of each (blk, k) group
    k_base = np.zeros((NBLK, K), np.int64)
    for b in range(NBLK):
        k_base[b] = blk_base[b] + np.concatenate([[0], np.cumsum(T_bk[b] * 128)[:-1]])

    # destination slot for every (sorted) edge
    grp_start = np.zeros(NCORES * NBLK * K + 1, np.int64)
    np.cumsum(counts.reshape(-1), out=grp_start[1:])
    rank = np.arange(len(gid_s)) - grp_start[gid_s]
    g_core = gid_s // (NBLK * K)
    g_blk = (gid_s // K) % NBLK
    g_k = gid_s % K
    slot = k_base[g_blk, g_k] + rank               # slot within the core's stream

    TT = SLOTS // 128
    Tmax = int(T_blk.max())
    IOTAR = np.tile(np.arange(128, dtype=np.float32), (128, Tmax)).astype(bf16)
    in_maps = []
    for c in range(NCORES):
        m = g_core == c
        sl = slot[m]
        idx_flat = np.zeros(SLOTS, np.int16)
        idx_flat[sl] = in_s[m].astype(np.int16)
        # per-slot out-loc / psi packed as (128, tile): S is built on-chip as
        # psi[e,t] * (iota == loc[e,t])
        e = sl % 128
        t = sl // 128
        LOC = np.zeros((128, TT), np.float32)
        PSIV = np.zeros((128, TT), np.float32)
        LOC[e, t] = loc_s[m].astype(np.float32)
        PSIV[e, t] = psi_s[m].astype(np.float32)
        LP = np.zeros((128, 2 * TT), np.float32)   # per blk: [loc T | psi T]
        for b in range(NBLK):
            t0, T = int(blk_base[b] // 128), int(T_blk[b])
            LP[:, 2 * t0:2 * t0 + T] = LOC[:, t0:t0 + T]
            LP[:, 2 * t0 + T:2 * (t0 + T)] = PSIV[:, t0:t0 + T]
        # wrap indices: per blk (T*128,) -> (T*8, 16) -> (16, T*8), tiled x8
        idx_cols = []
        for b in range(NBLK):
            seg = idx_flat[blk_base[b]:blk_base[b + 1]]
            idx_cols.append(seg.reshape(-1, 16).T)
        idx_w = np.tile(np.concatenate(idx_cols, axis=1), (8, 1))  # (128, SLOTS//16)
        in_maps.append({"XQ": XQ, "IDX": np.ascontiguousarray(idx_w),
                        "LP": LP.astype(bf16), "IOTAR": IOTAR})

    # block-diagonal weights: BW[k][j*64+c, j*64+o] = W[o,c,k]
    BW = np.zeros((K, 128, 128), np.float32)
    wt = weight.transpose(2, 1, 0)                 # (k, c, o)
    BW[:, :64, :64] = wt
    BW[:, 64:, 64:] = wt
    BWp = np.ascontiguousarray(BW.transpose(1, 0, 2).reshape(128, K * 128)).astype(bf16)
    for m in in_maps:
        m["BW"] = BWp

    return in_maps, T_bk, T_blk, blk_base, SLOTS


def _build(T_bk, T_blk, blk_base, SLOTS):
    """Emit the Bass/Tile program (identical for all cores)."""
    f32, bf16, i16 = mybir.dt.float32, mybir.dt.bfloat16, mybir.dt.int16

    # The SWDGE descriptor ring holds 1024 descriptors (ucode-fixed; resizing
    # the carveout is not honored on this runtime), so each dma_gather is
    # capped at 1024 indices. Rotating 4 queues keeps the Pool engine from
    # blocking on ring reclaim: enqueue is ~70 ns when the queue's ring is
    # free, and the SDMA drain proceeds in the background.
    TT = SLOTS // 128
    Tmax = int(T_blk.max())
    nc = bass.Bass(num_swdge_queues=4)
    XQ_d = nc.declare_dram_parameter("XQ", [N_IN, ROW], bf16, isOutput=False)
    IDX_d = nc.declare_dram_parameter("IDX", [128, SLOTS // 16], i16, isOutput=False)
    LP_d = nc.declare_dram_parameter("LP", [128, 2 * TT], bf16, isOutput=False)
    IOTAR_d = nc.declare_dram_parameter("IOTAR", [128, Tmax * 128], bf16, isOutput=False)
    BW_d = nc.declare_dram_parameter("BW", [128, K * 128], bf16, isOutput=False)
    Y_d = nc.declare_dram_parameter("Y", [NBLK, 128, 2 * 128], f32, isOutput=True)

    with TileContext(nc) as tc:
        with (
            tc.tile_pool(name="const", bufs=1) as cpool,
            tc.tile_pool(name="gp", bufs=4) as gpool,
            tc.tile_pool(name="sp", bufs=2) as spool,
            tc.tile_pool(name="ip", bufs=4) as ipool,
            tc.tile_pool(name="zc", bufs=18) as zcpool,
            tc.tile_pool(name="ys", bufs=2) as yspool,
            tc.tile_pool(name="zp", bufs=5, space="PSUM") as zpool,
            tc.tile_pool(name="yp", bufs=2, space="PSUM") as ypool,
        ):
            nc.gpsimd.load_library(library_config.mlp)
            bw = cpool.tile([128, K * 128], bf16)
            nc.sync.dma_start(bw[:], BW_d[:])
            iotar = cpool.tile([128, Tmax * 128], bf16)
            nc.sync.dma_start(iotar[:], IOTAR_d[:])
            gq = [0]                         # global gather-queue rotation

            reg_cache = {}

            def nreg(v):
                if v not in reg_cache:
                    reg_cache[v] = nc.gpsimd.to_reg(v)
                return reg_cache[v]

            for b in range(NBLK):
                T = int(T_blk[b])
                if T == 0:
                    continue
                c0 = int(blk_base[b])
                tb0 = c0 // 128
                idx_t = ipool.tile([128, T * 8], i16, tag="idx")
                lp_t = ipool.tile([128, 2 * T], bf16, tag="lp")
                nc.sync.dma_start(lp_t[:], LP_d[:, 2 * tb0:2 * (tb0 + T)])
                g_t = gpool.tile([128, T, ROW], bf16, tag="g")
                GT = 4                       # tiles per gather (1024 idx cap)
                for gi in range(0, T, GT):
                    ge = min(T, gi + GT)
                    # per-chunk idx slice: the gather fires as soon as its own
                    # indices land instead of waiting for the whole block's
                    nc.sync.dma_start(
                        idx_t[:, gi * 8:ge * 8],
                        IDX_d[:, c0 // 16 + gi * 8:c0 // 16 + ge * 8])
                    nc.gpsimd.dma_gather(
                        g_t[:, gi:ge, :], XQ_d[:],
                        idx_t[:, gi * 8:ge * 8],
                        num_idxs=(ge - gi) * 128, num_idxs_reg=nreg((ge - gi) * 128),
                        elem_size=ROW, queue_num=gq[0] % 4,
                    )
                    gq[0] += 1

                # build S on-chip: S[e, (t,n)] = psi[e,t] * (iota_n == loc[e,t])
                tmp_t = spool.tile([128, T * 128], bf16, tag="tmp")
                s_t = spool.tile([128, T * 128], bf16, tag="s")
                loc_b = lp_t[:, 0:T].to_broadcast([128, T, 128])
                psi_b = lp_t[:, T:2 * T].to_broadcast([128, T, 128])
                nc.vector.tensor_tensor(
                    out=tmp_t[:], in0=loc_b, in1=iotar[:, 0:T * 128],
                    op=mybir.AluOpType.is_equal)
                nc.vector.tensor_tensor(
                    out=s_t[:], in0=psi_b, in1=tmp_t[:],
                    op=mybir.AluOpType.mult)

                # PSUM accumulators: one bank per k-pair. start=True claims
                # the whole 2KB bank (zero region), so only the bank's FIRST
                # matmul starts and only its LAST stops; per-element
                # has_written turns the other first-touches into plain writes.
                z_tiles = [zpool.tile([128, 512], f32, tag="z", name=f"z{i}") for i in range(5)]
                t_starts = np.concatenate([[0], np.cumsum(T_bk[b])[:-1]])
                for pair in range(5):
                    ks = [k for k in (2 * pair, 2 * pair + 1)
                          if k < K and T_bk[b][k] > 0]
                    mms = [(k, int(t_starts[k]) + ti, half)
                           for k in ks for ti in range(int(T_bk[b][k]))
                           for half in range(2)]
                    for i, (k, t, half) in enumerate(mms):
                        sub = k % 2
                        nc.tensor.matmul(
                            out=z_tiles[pair][:, sub * 256 + half * 128:
                                              sub * 256 + (half + 1) * 128],
                            lhsT=g_t[:, t, half * 128:(half + 1) * 128],
                            rhs=s_t[:, t * 128:(t + 1) * 128],
                            start=(i == 0), stop=(i == len(mms) - 1),
                        )

                active = [k for k in range(K) if T_bk[b][k] > 0]
                zc_tiles = {}
                for j, k in enumerate(active):
                    pair, sub = k // 2, k % 2
                    zc = zcpool.tile([128, 256], bf16, tag="zc", name=f"zc{k}")
                    zc_tiles[k] = zc
                    nc.scalar.copy(zc[:], z_tiles[pair][:, sub * 256:(sub + 1) * 256])

                y_ps = ypool.tile([128, 256], f32, tag="y")
                for i, k in enumerate(active):
                    nc.tensor.matmul(
                        out=y_ps[:],
                        lhsT=bw[:, k * 128:(k + 1) * 128],
                        rhs=zc_tiles[k][:],
                        start=(i == 0), stop=(i == len(active) - 1),
                    )
                y_sb = yspool.tile([128, 256], f32, tag="ysb")
                nc.scalar.copy(y_sb[:], y_ps[:])
                nc.sync.dma_start(Y_d[b], y_sb[:])

    lower_extended_insts(nc)
    # this walrus build allows at most 1 sem-wait per instruction (2 on
    # event sems); split excess waits like Bacc does
    bass_rust.generate_event_semaphores(nc)
    return nc


def kernel(x, psi_idx, psi_vals, quadrature_weights, weight, bias):
    in_maps, T_bk, T_blk, blk_base, SLOTS = _prepare(
        x, psi_idx, psi_vals, quadrature_weights, weight
    )
    nc = _build(T_bk, T_blk, blk_base, SLOTS)
    core_ids = list(range(NCORES))
    res = run_bass_kernel_spmd(nc, in_maps, core_ids, trace=False)

    y = np.empty((B, COUT, N_OUT), np.float32)
    for c in core_ids:
        Yc = np.asarray(res.results[c]["Y"])          # (NBLK, 128, 256)
        # p = j*64+o, col = half*128+n, b = 2*half + j
        a = Yc.reshape(NBLK, 2, 64, 2, 128)           # (blk, j, o, half, n)
        a = a.transpose(3, 1, 2, 0, 4)                # (half, j, o, blk, n)
        y[:, :, c * PPC:(c + 1) * PPC] = a.reshape(B, COUT, PPC)
    y += bias.astype(np.float32)[None, :, None]
    return y
